# revision 1
# baseline (speedup 1.0000x reference)
"""BitLinear (RMSNorm + per-token int8 act fake-quant + ternary weight fake-quant
+ linear) Trainium2 Bass kernel, data-parallel over 8 NeuronCores.

Strategy
--------
Tokens (B*S = 32768) are sharded 8 ways (4096 tokens/core); W is replicated.
Host prep: the global weight-quant scale (one scalar statistic) and the
ternary weight quantization are computed on host with fp32 semantics matching
the reference; the ternary weights ship as fp8e4 (exact for {-1,0,1}, half
the DMA bytes) in k-major layout (the PE contracts over the partition dim of
both matmul operands).

Per core, per 128-token tile [128, 2048]:
  ACT:  sumsq via Square+accum_out -> rms = 1/sqrt(mean+eps) (Sqrt + DVE recip)
  DVE:  xn = (x * rms) * gamma  (scalar_tensor_tensor, exact reference order)
        absmax(xn) -> a = 127/(max+eps)
  ACT:  y = xn*a + C  (fma + magic constant C=1.5*2^23 gives exact RNE round)
  DVE:  q = y - C  -> bf16 ints in [-127, 127] (exactly representable)
  DMA:  xbar transpose q -> qT [k-part, kb, t]  (keeps the PE free)
  PE:   16 k-blocks x 4 out-groups matmuls, bf16 x fp8 (integer-exact, fp32
        psum; one PSUM bank per out-group, kt-major so weights load once per
        k-block and 4 matmuls reuse them)
  ACT:  out = psum * (1/(a*w_scale))  -> f32, DMA out

The matmul is numerically exact: quantized activations are integers
|I|<=127 (bf16-exact) and weights are ternary (fp8-exact), so products and
fp32 partial sums (<2^24) carry no rounding error.  The modeled time is
~478us/core: PE-bound at ~447us of back-to-back N=512 matmuls (the bf16
1 col/cycle roofline is 437us) plus ~22us pipeline fill and ~5us tail.
"""
import numpy as np
from contextlib import ExitStack

import concourse.bacc as bacc
import concourse.tile as tile
from concourse import mybir
from concourse.bass_utils import run_bass_kernel_spmd

F32 = mybir.dt.float32
BF16 = mybir.dt.bfloat16
FP8 = mybir.dt.float8e4
AL = mybir.AluOpType
AF = mybir.ActivationFunctionType
AX = mybir.AxisListType

B, S, DIN, DOUT = 4, 8192, 2048, 2048
NCORES = 8
TOK = B * S                  # 32768
TPC = TOK // NCORES          # 4096 tokens per core
NT = TPC // 128              # 32 token tiles per core
KB = DIN // 128              # 16 contraction blocks
OGW = 512                    # psum free dim per matmul
OG = DOUT // OGW             # 4 output groups

C_MAGIC = 12582912.0         # 1.5 * 2^23: fp32 +C/-C rounds to nearest int (RNE)
WCLIP = 1.4999999            # clip-before-round == round-then-clip to [-1,1]

_CACHE = {}


def _build():
    nc = bacc.Bacc("TRN2", target_bir_lowering=False, debug=False,
                   num_devices=NCORES)
    x_d = nc.declare_dram_parameter("x", [TPC, DIN], F32, isOutput=False)
    g_d = nc.declare_dram_parameter("gamma", [1, DIN], F32, isOutput=False)
    wq_d = nc.declare_dram_parameter("wq", [DIN, DOUT], FP8, isOutput=False)
    sc_d = nc.declare_dram_parameter("sc", [1, 1], F32, isOutput=False)
    o_d = nc.declare_dram_parameter("out", [TPC, DOUT], F32, isOutput=True)

    with tile.TileContext(nc) as tc:
        with ExitStack() as ctx:
            cst = ctx.enter_context(tc.tile_pool(name="cst", bufs=1))
            wqp = ctx.enter_context(tc.tile_pool(name="wqp", bufs=1))
            xp = ctx.enter_context(tc.tile_pool(name="xp", bufs=4))
            sp = ctx.enter_context(tc.tile_pool(name="sp", bufs=3))
            qp = ctx.enter_context(tc.tile_pool(name="qp", bufs=3))
            qtp = ctx.enter_context(tc.tile_pool(name="qtp", bufs=3))
            op = ctx.enter_context(tc.tile_pool(name="op", bufs=3))
            st = ctx.enter_context(tc.tile_pool(name="st", bufs=4))
            pso = ctx.enter_context(tc.tile_pool(name="pso", bufs=2, space="PSUM"))

            # ---- constants + DMA issue order ----
            # The DMA engine pool drains roughly in issue order, so prioritize:
            # first x tile, gamma (needed by the first normalize), then the
            # ternary weight chunks (needed progressively by the first tile's
            # k-block matmuls), then the x-tile stream.
            NPRE = 2  # x tiles DMA'd ahead of the weight chunks
            xpre = [xp.tile([128, DIN], F32, name="xt", tag="xtile")
                    for _ in range(NPRE)]
            nc.sync.dma_start(out=xpre[0], in_=x_d[0:128, :])
            gam = cst.tile([128, DIN], F32, name="gam")
            nc.sync.dma_start(out=gam, in_=g_d[:].to_broadcast((128, DIN)))
            scb = cst.tile([128, 1], F32, name="scb")
            nc.sync.dma_start(out=scb, in_=sc_d[:].to_broadcast((128, 1)))
            inv_b = scb[:, 0:1]    # 1/w_scale
            cmag = cst.tile([128, 1], F32, name="cmag")
            nc.vector.memset(cmag, C_MAGIC)
            ceps = cst.tile([128, 1], F32, name="ceps")
            nc.vector.memset(ceps, 1e-6)
            warmt = cst.tile([128, 1], F32, name="warmt")
            nc.scalar.activation(out=warmt, in_=cmag, func=AF.Square)
            nc.scalar.activation(out=warmt, in_=cmag, func=AF.Sqrt)

            for it in range(1, NPRE):
                nc.sync.dma_start(out=xpre[it],
                                  in_=x_d[it * 128:(it + 1) * 128, :])

            # ---- ternary weights (host-quantized fp8, exact for
            # {-1,0,1}), k-major in SBUF; chunked per k-block and issued in
            # two groups so tile 0's transpose DMA isn't queued behind them ----
            wq = wqp.tile([128, KB, DOUT], FP8, name="wq")

            def dma_wq(kt):
                nc.sync.dma_start(out=wq[:, kt, :],
                                  in_=wq_d[kt * 128:(kt + 1) * 128, :])
            for kt in range(3):
                dma_wq(kt)

            # ---- token tiles ----
            for it in range(NT):
                split = 1
                HW_ = DIN // split
                parts = [(h * HW_, HW_) for h in range(split)]

                if it < NPRE:
                    xt = xpre[it]
                else:
                    xt = xp.tile([128, DIN], F32, name="xt", tag="xtile")
                    nc.sync.dma_start(out=xt,
                                      in_=x_d[it * 128:(it + 1) * 128, :])

                # sum of squares of raw x (per token)
                scr = sp.tile([128, DIN], F32, name="scr")
                sshs = []
                for h, (o0, w) in enumerate(parts):
                    ssh = st.tile([128, 1], F32, name=f"ss{h}", tag=f"ss{h}")
                    nc.scalar.activation(out=scr[:, o0:o0 + w],
                                         in_=xt[:, o0:o0 + w], func=AF.Square,
                                         accum_out=ssh)
                    sshs.append(ssh)
                ss = sshs[0]
                if split > 1:
                    ss = st.tile([128, 1], F32, name="ss")
                    nc.vector.tensor_tensor(out=ss, in0=sshs[0], in1=sshs[1],
                                            op=AL.add)
                # rms = 1/sqrt(ss/DIN + 1e-6)
                sqv = st.tile([128, 1], F32, name="sqv")
                nc.scalar.activation(out=sqv, in_=ss, func=AF.Sqrt, bias=ceps,
                                     scale=1.0 / DIN)
                rms = st.tile([128, 1], F32, name="rms")
                nc.vector.reciprocal(out=rms, in_=sqv)

                # xn = (x * rms) * gamma   (in-place, exact reference order)
                mxhs = []
                for h, (o0, w) in enumerate(parts):
                    nc.vector.scalar_tensor_tensor(
                        out=xt[:, o0:o0 + w], in0=xt[:, o0:o0 + w], scalar=rms,
                        in1=gam[:, o0:o0 + w], op0=AL.mult, op1=AL.mult)
                    mxh = st.tile([128, 1], F32, name=f"mx{h}", tag=f"mx{h}")
                    nc.vector.reduce_max(out=mxh, in_=xt[:, o0:o0 + w],
                                         axis=AX.X, apply_absolute_value=True)
                    mxhs.append(mxh)
                mx = mxhs[0]
                if split > 1:
                    mx = st.tile([128, 1], F32, name="mx")
                    nc.vector.tensor_tensor(out=mx, in0=mxhs[0], in1=mxhs[1],
                                            op=AL.max)

                # a = 127 / (absmax(xn) + 1e-5)
                d = st.tile([128, 1], F32, name="d")
                nc.vector.tensor_scalar(out=d, in0=mx, scalar1=1e-5,
                                        scalar2=None, op0=AL.add)
                rcd = st.tile([128, 1], F32, name="rcd")
                nc.vector.reciprocal(out=rcd, in_=d)
                a = st.tile([128, 1], F32, name="a")
                nc.vector.tensor_scalar(out=a, in0=rcd, scalar1=127.0,
                                        scalar2=None, op0=AL.mult)
                # s3 = (1/a) * (1/ws)
                ra = st.tile([128, 1], F32, name="ra")
                nc.vector.reciprocal(out=ra, in_=a)
                s3 = st.tile([128, 1], F32, name="s3")
                nc.vector.tensor_scalar(out=s3, in0=ra, scalar1=inv_b,
                                        scalar2=None, op0=AL.mult)

                # y = xn*a + C (ACT fma: exact RNE round); q = y - C -> bf16;
                # transpose to contraction-major via the DMA xbar engine
                # (keeps the PE free; separate queue family from bulk copies)
                q = qp.tile([128, DIN], BF16, name="q")
                qt = qtp.tile([128, KB, 128], BF16, name="qt")
                for h, (o0, w) in enumerate(parts):
                    nc.scalar.activation(out=scr[:, o0:o0 + w],
                                         in_=xt[:, o0:o0 + w],
                                         func=AF.Identity, bias=cmag, scale=a)
                    nc.vector.tensor_scalar(out=q[:, o0:o0 + w],
                                            in0=scr[:, o0:o0 + w],
                                            scalar1=C_MAGIC, scalar2=None,
                                            op0=AL.subtract)
                    kb0, kbw = o0 // 128, w // 128
                    nc.scalar.dma_start_transpose(qt[:, kb0:kb0 + kbw, :],
                                                  q[:, o0:o0 + w])
                if it == 0:
                    for kt in range(3, KB):
                        dma_wq(kt)

                # matmul: out[t, o] = sum_k qT[k, t] * wq[k, o]
                # last tile runs og-major so its evacuation overlaps the MMs
                pos = [pso.tile([128, OGW], F32, name=f"po{og}", tag=f"po{og}")
                       for og in range(OG)]
                ot = op.tile([128, DOUT], F32, name="ot")
                last = it == NT - 1
                if True:
                    for kt in range(KB):
                        lhsT = qt[:, kt, :]
                        for og in range(OG):
                            nc.tensor.matmul(
                                pos[og], lhsT=lhsT,
                                rhs=wq[:, kt, og * OGW:(og + 1) * OGW],
                                start=(kt == 0), stop=(kt == KB - 1))
                    for og in range(OG):
                        nc.scalar.mul(out=ot[:, og * OGW:(og + 1) * OGW],
                                      in_=pos[og], mul=s3)
                        if last:
                            nc.sync.dma_start(
                                out=o_d[it * 128:(it + 1) * 128,
                                        og * OGW:(og + 1) * OGW],
                                in_=ot[:, og * OGW:(og + 1) * OGW])
                    if not last:
                        nc.sync.dma_start(out=o_d[it * 128:(it + 1) * 128, :],
                                          in_=ot)

    nc.compile()
    return nc


def kernel(x, gamma, W):
    x = np.asarray(x, dtype=np.float32)
    gamma = np.asarray(gamma, dtype=np.float32)
    W = np.asarray(W, dtype=np.float32)

    # host prep: ternary-quantized k-major weights + the global scale, using
    # fp32 semantics matching the reference:
    #   w_scale = 1/(mean|W| + 1e-5);  w_q = clip(round(W*w_scale), -1, 1)
    import ml_dtypes
    m = np.float32(np.abs(W).astype(np.float64).mean())
    denom = np.float32(m + np.float32(1e-5))
    ws = np.float32(np.float32(1.0) / denom)
    inv_ws = np.float32(np.float32(1.0) / ws)
    wqh = np.clip(np.rint((W * ws).astype(np.float32)), -1.0, 1.0)
    wq = np.ascontiguousarray(wqh.T.astype(ml_dtypes.float8_e4m3))
    sc = np.array([[inv_ws]], dtype=np.float32)
    g2 = gamma.reshape(1, DIN)

    if "nc" not in _CACHE:
        _CACHE["nc"] = _build()
    nc = _CACHE["nc"]

    xf = x.reshape(TOK, DIN)
    in_maps = [
        {"x": xf[c * TPC:(c + 1) * TPC], "gamma": g2, "wq": wq, "sc": sc}
        for c in range(NCORES)
    ]
    res = run_bass_kernel_spmd(nc, in_maps, list(range(NCORES)))
    out = np.concatenate([res.results[c]["out"] for c in range(NCORES)], axis=0)
    return out.reshape(B, S, DOUT)


if __name__ == "__main__":
    rng = np.random.default_rng(0)
    x = rng.standard_normal((B, S, DIN), dtype=np.float32)
    gamma = np.ones((DIN,), dtype=np.float32)
    bound = 1.0 / np.sqrt(DIN)
    W = rng.uniform(-bound, bound, (DOUT, DIN)).astype(np.float32)
    out = kernel(x, gamma, W)
    print("out", out.shape, out.dtype, float(np.abs(out).mean()))



# revision 2
# speedup vs baseline: 1.2659x; 1.2659x over previous
"""BitLinear (RMSNorm + per-token int8 act fake-quant + ternary weight fake-quant
+ linear) Trainium2 Bass kernel, data-parallel over 8 NeuronCores.

Strategy (v2: exact-integer DoubleRow fp8 matmul)
-------------------------------------------------
Tokens (B*S = 32768) are sharded 8 ways (4096 tokens/core); W is replicated.
Host prep: ternary weight quantization (exact fp8 {-1,0,1}, k-major) and the
global weight scale, as in the reference; x ships as fp16 (RNE), which
perturbs the int8 fake-quant rounding of ~0.4% of elements by +-1 step and
keeps the end-to-end output error ~4e-3 (tolerance 2e-2).

Per core, per 128-token tile [128, 2048]:
  ACT:  sumsq via Square+accum -> rms chain; y = x*rs + C (magic-C RNE round)
  DVE:  absmax; q = y - C -> bf16 integers I_x in [-127,127]
  DMA:  xbar-transpose q -> qT (k-major)
  DVE:  hiT = fp8(qT)  (RNE fp8e4m3: exact multiples-of-8 above 64, etc.)
  POOL: loT = qT - hiT -> fp8 (integer residual in [-4,4], fp8-exact)
  PE:   DoubleRow fp8 matmuls: pair dim = (hi, lo) against the SAME ternary
        weight block (stride-0 broadcast rhs).  Each instruction computes
        hiT.T @ W + loT.T @ W = I_x.T @ W exactly (fp32 psum), at HALF the
        bf16 cycle cost (cost model: 0.5 cycles/moving-row).
  ACT/DVE: out = psum * s3 -> bf16, DMA out (host upcasts to f32).

The matmul is numerically exact: hi/lo/w are fp8-exact integers, products
and fp32 partial sums (<2^24) carry no rounding error.  PE per tile:
8 out-groups x 16 k-blocks x 128 cycles = 16384 cycles (6.8us) -- 2x the
bf16 roofline.  DMA (serial in the cost model) is ~6.9us/tile: x fp16
(1.46us) + q transpose (3.58us) + out bf16 (1.46us) + W amortized.
"""
import numpy as np
from contextlib import ExitStack

import concourse.bacc as bacc
import concourse.tile as tile
from concourse import mybir
from concourse.bass_utils import run_bass_kernel_spmd

F32 = mybir.dt.float32
F16 = mybir.dt.float16
BF16 = mybir.dt.bfloat16
FP8 = mybir.dt.float8e4
AL = mybir.AluOpType
AF = mybir.ActivationFunctionType
AX = mybir.AxisListType
DR = mybir.MatmulPerfMode.DoubleRow

B, S, DIN, DOUT = 4, 8192, 2048, 2048
NCORES = 8
TOK = B * S                  # 32768
TPC = TOK // NCORES          # 4096 tokens per core
NT = TPC // 128              # 32 token tiles per core
KB = DIN // 128              # 16 contraction blocks
OGW = 256                    # moving pair free = 2*256 = 512 (max)
OG = DOUT // OGW             # 8 output groups
ACT_OGS = 5                  # out-groups evacuated on ACT (rest on DVE)

C_MAGIC = 12582912.0         # 1.5 * 2^23: fp32 +C/-C rounds to nearest (RNE)

_CACHE = {}


def _build(gamma_is_one: bool):
    nc = bacc.Bacc("TRN2", target_bir_lowering=False, debug=False,
                   num_devices=NCORES)
    x_d = nc.declare_dram_parameter("x", [TPC, DIN], F16, isOutput=False)
    g_d = nc.declare_dram_parameter("gamma", [1, DIN], F32, isOutput=False)
    wq_d = nc.declare_dram_parameter("wq", [DIN, DOUT], FP8, isOutput=False)
    sc_d = nc.declare_dram_parameter("sc", [1, 1], F32, isOutput=False)
    o_d = nc.declare_dram_parameter("out", [TPC, DOUT], BF16, isOutput=True)

    with tile.TileContext(nc) as tc:
        with ExitStack() as ctx:
            cst = ctx.enter_context(tc.tile_pool(name="cst", bufs=1))
            wqp = ctx.enter_context(tc.tile_pool(name="wqp", bufs=1))
            xp = ctx.enter_context(tc.tile_pool(name="xp", bufs=4))
            sp = ctx.enter_context(tc.tile_pool(name="sp", bufs=2))
            yp = ctx.enter_context(tc.tile_pool(name="yp", bufs=2))
            qp = ctx.enter_context(tc.tile_pool(name="qp", bufs=2))
            qtp = ctx.enter_context(tc.tile_pool(name="qtp", bufs=2))
            prp = ctx.enter_context(tc.tile_pool(name="prp", bufs=2))
            op = ctx.enter_context(tc.tile_pool(name="op", bufs=3))
            st = ctx.enter_context(tc.tile_pool(name="st", bufs=4))
            pso = ctx.enter_context(tc.tile_pool(name="pso", bufs=1,
                                                 space="PSUM"))

            # ---- constants; DMA issue order: first x tiles, then W ----
            NPRE = 3
            xpre = [xp.tile([128, DIN], F16, name="xt", tag="xtile")
                    for _ in range(NPRE)]
            nc.sync.dma_start(out=xpre[0], in_=x_d[0:128, :])
            scb = cst.tile([128, 1], F32, name="scb")
            nc.sync.dma_start(out=scb, in_=sc_d[:].to_broadcast((128, 1)))
            c0 = scb[:, 0:1]             # (1/ws)/127
            if not gamma_is_one:
                gam = cst.tile([128, DIN], F32, name="gam")
                nc.sync.dma_start(out=gam, in_=g_d[:].to_broadcast((128, DIN)))
            cmag = cst.tile([128, 1], F32, name="cmag")
            nc.vector.memset(cmag, C_MAGIC)
            ceps = cst.tile([128, 1], F32, name="ceps")
            nc.vector.memset(ceps, 1e-6)
            for it in range(1, NPRE):
                nc.sync.dma_start(out=xpre[it],
                                  in_=x_d[it * 128:(it + 1) * 128, :])

            # ---- ternary weights, k-major fp8 [128, KB, DOUT] ----
            wq = wqp.tile([128, KB, DOUT], FP8, name="wq")
            for kt in range(KB):
                nc.sync.dma_start(out=wq[:, kt, :],
                                  in_=wq_d[kt * 128:(kt + 1) * 128, :])

            # ---- token tiles ----
            for it in range(NT):
                if it < NPRE:
                    xt = xpre[it]
                else:
                    xt = xp.tile([128, DIN], F16, name="xt", tag="xtile")
                    nc.sync.dma_start(out=xt,
                                      in_=x_d[it * 128:(it + 1) * 128, :])

                # sum of squares of raw x (per token) -> ss
                scr = sp.tile([128, DIN], BF16, name="scr")
                ss = st.tile([128, 1], F32, name="ss", tag="ss")
                nc.scalar.activation(out=scr, in_=xt, func=AF.Square,
                                     accum_out=ss)

                if gamma_is_one:
                    ut = xt
                else:
                    ut = sp.tile([128, DIN], F32, name="ut", tag="ut")
                    nc.vector.tensor_tensor(out=ut, in0=xt, in1=gam,
                                            op=AL.mult)

                # absmax of u = x*gamma (per token)
                mx = st.tile([128, 1], F32, name="mx", tag="mx")
                nc.vector.reduce_max(out=mx, in_=ut, axis=AX.X,
                                     apply_absolute_value=True)

                # rms = 1/sqrt(ss/DIN + 1e-6)
                sq = st.tile([128, 1], F32, name="sq", tag="sq")
                nc.scalar.activation(out=sq, in_=ss, func=AF.Sqrt, bias=ceps,
                                     scale=1.0 / DIN)
                rms = st.tile([128, 1], F32, name="rms", tag="rms")
                nc.vector.reciprocal(out=rms, in_=sq)

                # d = mx*rms + 1e-5  (= absmax(xn) + eps)
                d = st.tile([128, 1], F32, name="d", tag="d")
                nc.gpsimd.tensor_scalar(out=d, in0=mx, scalar1=rms,
                                        scalar2=1e-5, op0=AL.mult, op1=AL.add)
                rd = st.tile([128, 1], F32, name="rd", tag="rd")
                nc.vector.reciprocal(out=rd, in_=d)
                # rs = rms * a = rms * 127/d
                rs = st.tile([128, 1], F32, name="rs", tag="rs")
                nc.gpsimd.tensor_scalar(out=rs, in0=rd, scalar1=rms,
                                        scalar2=127.0, op0=AL.mult,
                                        op1=AL.mult)
                # s3 = (1/a) * (1/ws) = d * ((1/ws)/127)
                s3 = st.tile([128, 1], F32, name="s3", tag="s3")
                nc.gpsimd.tensor_scalar(out=s3, in0=d, scalar1=c0,
                                        scalar2=None, op0=AL.mult)

                # y = u*rs + C  (ACT fma; adding C rounds to nearest int)
                yb = yp.tile([128, DIN], F32, name="yb")
                nc.scalar.activation(out=yb, in_=ut, func=AF.Identity,
                                     bias=cmag, scale=rs)
                # q = y - C -> bf16 integers in [-127, 127]
                qb = qp.tile([128, DIN], BF16, name="qb")
                nc.vector.tensor_scalar(out=qb, in0=yb, scalar1=C_MAGIC,
                                        scalar2=None, op0=AL.subtract)

                # k-major transpose via the DMA xbar (ACT hwdge queue)
                qt = qtp.tile([128, KB, 128], BF16, name="qt")
                nc.scalar.dma_start_transpose(qt[:, :, :], qb)

                # hi = fp8(q) (RNE), lo = q - hi in [-4,4] -> pair [k,2,t]
                pr = prp.tile([128, KB, 2, 128], FP8, name="pr")
                nc.vector.tensor_scalar(out=pr[:, :, 0, :], in0=qt,
                                        scalar1=1.0, scalar2=None,
                                        op0=AL.mult)
                nc.gpsimd.tensor_tensor(out=pr[:, :, 1, :], in0=qt,
                                        in1=pr[:, :, 0, :], op=AL.subtract)

                # DoubleRow matmuls, out-group major (evac overlaps matmuls)
                ot = op.tile([128, DOUT], BF16, name="ot")
                for og in range(OG):
                    po = pso.tile([128, OGW], F32, name=f"po{og}",
                                  tag=f"po{og}")
                    for kt in range(KB):
                        rhs = wq[:, kt, og * OGW:(og + 1) * OGW]
                        rhs = rhs.unsqueeze(1).broadcast_to((128, 2, OGW))
                        nc.tensor.matmul(po, lhsT=pr[:, kt, :, :], rhs=rhs,
                                         start=(kt == 0), stop=(kt == KB - 1),
                                         perf_mode=DR)
                    osl = ot[:, og * OGW:(og + 1) * OGW]
                    if og < ACT_OGS:
                        nc.scalar.mul(out=osl, in_=po, mul=s3)
                    else:
                        nc.vector.tensor_scalar(out=osl, in0=po, scalar1=s3,
                                                scalar2=None, op0=AL.mult)
                nc.sync.dma_start(out=o_d[it * 128:(it + 1) * 128, :], in_=ot)

    nc.compile()
    return nc


def kernel(x, gamma, W):
    x = np.asarray(x, dtype=np.float32)
    gamma = np.asarray(gamma, dtype=np.float32)
    W = np.asarray(W, dtype=np.float32)

    # host prep: ternary-quantized k-major weights + the global scale, fp32
    # semantics matching the reference:
    #   w_scale = 1/(mean|W| + 1e-5);  w_q = clip(round(W*w_scale), -1, 1)
    import ml_dtypes
    m = np.float32(np.abs(W).astype(np.float64).mean())
    denom = np.float32(m + np.float32(1e-5))
    ws = np.float32(np.float32(1.0) / denom)
    inv_ws = np.float32(np.float32(1.0) / ws)
    wqh = np.clip(np.rint((W * ws).astype(np.float32)), -1.0, 1.0)
    wq = np.ascontiguousarray(wqh.T.astype(ml_dtypes.float8_e4m3))
    sc = np.array([[np.float32(inv_ws / np.float32(127.0))]], dtype=np.float32)
    g2 = gamma.reshape(1, DIN)

    gamma_is_one = bool(np.all(gamma == np.float32(1.0)))
    key = ("nc", gamma_is_one)
    if key not in _CACHE:
        _CACHE[key] = _build(gamma_is_one)
        _CACHE["nc"] = _CACHE[key]
    nc = _CACHE[key]

    xf = np.ascontiguousarray(x.reshape(TOK, DIN)).astype(np.float16)
    in_maps = [
        {"x": xf[c * TPC:(c + 1) * TPC], "gamma": g2, "wq": wq, "sc": sc}
        for c in range(NCORES)
    ]
    res = run_bass_kernel_spmd(nc, in_maps, list(range(NCORES)))
    out = np.concatenate([res.results[c]["out"].astype(np.float32)
                          for c in range(NCORES)], axis=0)
    return out.reshape(B, S, DOUT)


if __name__ == "__main__":
    rng = np.random.default_rng(0)
    x = rng.standard_normal((B, S, DIN), dtype=np.float32)
    gamma = np.ones((DIN,), dtype=np.float32)
    bound = 1.0 / np.sqrt(DIN)
    W = rng.uniform(-bound, bound, (DOUT, DIN)).astype(np.float32)
    out = kernel(x, gamma, W)
    print("out", out.shape, out.dtype, float(np.abs(out).mean()))
